# revision 31
# baseline (speedup 1.0000x reference)
"""NetVLAD Trainium2 kernel: 8-core data-parallel over batch.

Self-contained: builds a Bass/Tile program once, shards the batch (16 items
-> 2 per core), runs SPMD on 8 NeuronCores, gathers full output.
"""
import os
import sys

sys.path.insert(0, "/opt/trn_rl_repo")

import numpy as np
import ml_dtypes

import concourse.bass as bass  # noqa: F401
import concourse.mybir as mybir
from concourse import bacc, tile
from concourse.bass_utils import run_bass_kernel_spmd

EPS_BN = 1e-5
N_CORES = 8
BS, T, C, K = 16, 8192, 128, 64
PB = BS // N_CORES          # batch items per core
NT = T // 128               # 64 t-tiles of 128
NCH = T // 512              # 16 chunks of 512
NBK = NT // 8               # 8 "banks" of 8 tiles
NG = 4                      # norm groups (16 tiles each)
MASK_NEG = -200.0           # exp(h2 + MASK_NEG) == 0.0 exactly in f32

f32 = mybir.dt.float32
bf16 = mybir.dt.bfloat16
AL = mybir.AluOpType
AF = mybir.ActivationFunctionType
bfloat16 = ml_dtypes.bfloat16

_CACHE = {}


def _build_program(stages=6):
    nc = bacc.Bacc("TRN2", target_bir_lowering=False, debug=False,
                   num_devices=N_CORES)

    # x host-reshaped to [PB, 128(p), NT, 128(c)] so each SBUF partition's
    # data is one contiguous 32KB run in DRAM.
    x_d = nc.dram_tensor("x", [PB, 128, NT, 128], f32, kind="ExternalInput")
    lb_d = nc.dram_tensor("lb", [PB, NBK, 10, 128], bf16, kind="ExternalInput")
    invm_d = nc.dram_tensor("invm", [PB, 128, NT], bf16, kind="ExternalInput")
    d3_d = nc.dram_tensor("d3", [3, 128, 128], bf16, kind="ExternalInput")
    i128_d = nc.dram_tensor("i128", [128, 128], bf16, kind="ExternalInput")
    w2t_d = nc.dram_tensor("w2t", [C, K], bf16, kind="ExternalInput")
    rb_d = nc.dram_tensor("rb", [10, 512], bf16, kind="ExternalInput")
    b1_d = nc.dram_tensor("b1", [C, 1], f32, kind="ExternalInput")
    cent_d = nc.dram_tensor("cent", [K, C], f32, kind="ExternalInput")
    ones_d = nc.dram_tensor("ones64", [64, 1], f32, kind="ExternalInput")
    onesr_d = nc.dram_tensor("ones1x64", [1, 64], f32, kind="ExternalInput")
    out_d = nc.dram_tensor("out", [PB, K, C], f32, kind="ExternalOutput")

    with tile.TileContext(nc) as tc:
        with (
            tc.tile_pool(name="consts", bufs=1) as pc,
            tc.tile_pool(name="x", bufs=2) as px,
            tc.tile_pool(name="xn", bufs=2) as pxn,
            tc.tile_pool(name="xnT", bufs=2) as pxt,
            tc.tile_pool(name="h", bufs=1) as ph,
            tc.tile_pool(name="a", bufs=2) as pa,
            tc.tile_pool(name="small", bufs=4) as psm,
            tc.tile_pool(name="tail", bufs=2) as ptl,
            tc.tile_pool(name="ps_t", bufs=2, space="PSUM") as ps_t,
            tc.tile_pool(name="ps_u", bufs=2, space="PSUM") as ps_u,
            tc.tile_pool(name="ps_c2", bufs=2, space="PSUM") as ps_c2,
            tc.tile_pool(name="ps_vl", bufs=2, space="PSUM") as ps_vl,
        ):
            # ---- constants (issued on gpsimd queue to keep sync free) ----
            d3 = pc.tile([128, 3, 128], bf16)
            i128 = pc.tile([128, 128], bf16)
            w2t = pc.tile([C, K], bf16)
            rb = pc.tile([10, 512], bf16)
            b1 = pc.tile([C, 1], f32)
            cent = pc.tile([K, C], f32)
            lb = pc.tile([10, PB, NBK, 128], bf16)
            invm = pc.tile([128, PB, NT], bf16)
            o64 = pc.tile([64, 1], f32)
            o1x64 = pc.tile([1, 64], f32)
            eps24 = pc.tile([128, 1], f32)
            nc.vector.memset(eps24[:], 1e-24)
            nc.gpsimd.dma_start(d3[:], d3_d.ap().rearrange("d p c -> p d c"))
            nc.gpsimd.dma_start(i128[:], i128_d.ap())
            nc.gpsimd.dma_start(w2t[:], w2t_d.ap())
            nc.gpsimd.dma_start(rb[:], rb_d.ap())
            nc.gpsimd.dma_start(b1[:], b1_d.ap())
            nc.gpsimd.dma_start(cent[:], cent_d.ap())
            nc.gpsimd.dma_start(o64[:], ones_d.ap())
            nc.gpsimd.dma_start(o1x64[:], onesr_d.ap())
            nc.gpsimd.dma_start(lb[:], lb_d.ap().rearrange("n g r p -> r n g p"))
            nc.gpsimd.dma_start(invm[:], invm_d.ap().rearrange("n p t -> p n t"))

            GT = NT // NG   # 16 tiles per norm group

            for n in range(PB):
                # ---- load x: 16 DMAs of 4 tiles each ----
                x_sb = px.tile([128, NT, 128], f32, tag="x")
                for g in range(16):
                    nc.sync.dma_start(
                        x_sb[:, 4 * g:4 * g + 4, :],
                        x_d.ap()[n, :, 4 * g:4 * g + 4, :])

                # ---- norms + normalize, per group of 16 tiles ----
                stats = psm.tile([128, NT, 6], f32, tag="stats")
                rinv = psm.tile([128, NT], f32, tag="rinv")
                xn = pxn.tile([128, NT, 130], bf16, tag="xn")
                nc.vector.memset(xn[:, :, 128:130], 0.0)
                nc.vector.memset(xn[:, :, 128:129], 1.0)
                for grp in range(NG):
                    t0 = GT * grp
                    for i in range(t0, t0 + GT):
                        nc.vector.bn_stats(stats[:, i, :], x_sb[:, i, :])
                    gs_ = slice(t0, t0 + GT)
                    msq = psm.tile([128, GT, 2], f32, tag="msq")
                    nc.vector.tensor_tensor(
                        msq[:], stats[:, gs_, 1:5:3], stats[:, gs_, 1:5:3],
                        AL.mult)
                    msum = psm.tile([128, GT], f32, tag="msum")
                    nc.vector.reduce_sum(msum[:], msq[:],
                                         axis=mybir.AxisListType.X)
                    m2s = psm.tile([128, GT], f32, tag="m2s")
                    nc.vector.reduce_sum(m2s[:], stats[:, gs_, 2:6:3],
                                         axis=mybir.AxisListType.X)
                    s_all = psm.tile([128, GT], f32, tag="s_all")
                    nc.vector.scalar_tensor_tensor(
                        s_all[:], msum[:], 64.0, m2s[:], AL.mult, AL.add)
                    sn = psm.tile([128, GT], f32, tag="sn")
                    nc.scalar.activation(sn[:], s_all[:], AF.Sqrt,
                                         bias=eps24[:, 0:1])
                    nc.vector.reciprocal(rinv[:, gs_], sn[:])
                    rb_ = rinv[:, gs_].unsqueeze(-1).broadcast_to(
                        (128, GT, 128))
                    nc.vector.tensor_tensor(
                        xn[:, gs_, 0:128], x_sb[:, gs_, :], rb_, AL.mult)

                if stages == 2:
                    vdbg = ptl.tile([64, C], f32, tag="vdbg")
                    nc.vector.tensor_copy(vdbg[:], xn[0:64, 0, 0:128])
                    nc.sync.dma_start(out_d.ap()[n], vdbg[:])
                    continue

                # ---- transpose xn -> xnT bf16 [C,T] with halo ----
                xnT = pxt.tile([128, 2 + T + 2], bf16, tag="xnT")
                nc.vector.memset(xnT[:, 0:2], 0.0)
                nc.vector.memset(xnT[:, 2 + T:], 0.0)
                for q in range(NCH):
                    tp = ps_t.tile([128, 512], bf16, tag="tp")
                    for j in range(4):
                        i = 4 * q + j
                        nc.tensor.transpose(
                            tp[:, 128 * j:128 * (j + 1)],
                            xn[:, i, 0:128], i128[:])
                    nc.scalar.activation(
                        xnT[:, 2 + 512 * q:2 + 512 * (q + 1)], tp[:], AF.Copy)

                if stages == 3:
                    vdbg = ptl.tile([64, C], f32, tag="vdbg")
                    nc.vector.tensor_copy(vdbg[:], xnT[0:64, 0:128])
                    nc.sync.dma_start(out_d.ap()[n], vdbg[:])
                    continue

                # ---- conv1 (3 diag matmuls) + bias + relu -> h bf16 ----
                h = ph.tile([128, T], bf16, tag="h")
                for q in range(NCH):
                    up = ps_u.tile([128, 512], f32, tag="up")
                    for d in range(3):
                        off = 2 + 512 * q + (d - 1)
                        nc.tensor.matmul(
                            up[:], d3[:, d, :], xnT[:, off:off + 512],
                            start=(d == 0), stop=(d == 2))
                    nc.scalar.activation(
                        h[:, 512 * q:512 * (q + 1)], up[:], AF.Relu,
                        bias=b1[:, 0:1])

                if stages == 4:
                    vdbg = ptl.tile([64, C], f32, tag="vdbg")
                    nc.vector.tensor_copy(vdbg[:], h[0:64, 0:128])
                    nc.sync.dma_start(out_d.ap()[n], vdbg[:])
                    continue

                # ---- conv2 + bias/mask matmul -> exp -> softmax ----
                a_sb = pa.tile([128, NT, 66], bf16, tag="a")
                nc.vector.tensor_copy(a_sb[:, :, 64:65],
                                      invm[:, n, :].unsqueeze(-1))
                z_all = psm.tile([128, NT], f32, tag="z_all")
                rz = psm.tile([128, NT], f32, tag="rz")
                rzb = psm.tile([128, NT], bf16, tag="rzb")
                for g in range(NBK):
                    c2 = ps_c2.tile([128, 512], f32, tag="c2")
                    nc.tensor.matmul(c2[:], lb[:, n, g, :], rb[:],
                                     start=True, stop=False,
                                     skip_group_check=True)
                    for j in range(8):
                        i = 8 * g + j
                        nc.tensor.matmul(
                            c2[:, 64 * j:64 * (j + 1)],
                            h[:, 128 * i:128 * (i + 1)], w2t[:],
                            start=False, stop=True, skip_group_check=True)
                    gsl = slice(8 * g, 8 * g + 8)
                    nc.scalar.activation(a_sb[:, gsl, 0:64], c2[:], AF.Exp)
                    nc.vector.reduce_sum(z_all[:, gsl], a_sb[:, gsl, 0:64],
                                         axis=mybir.AxisListType.X)
                    zp = psm.tile([128, 8], f32, tag="zp")
                    nc.vector.tensor_scalar(zp[:], z_all[:, gsl], 1e-30,
                                            None, AL.add)
                    nc.vector.reciprocal(rz[:, gsl], zp[:])
                    nc.vector.tensor_copy(rzb[:, gsl], rz[:, gsl])
                    rzbb = rzb[:, gsl].unsqueeze(-1).broadcast_to(
                        (128, 8, 64))
                    nc.vector.tensor_tensor(
                        a_sb[:, gsl, 0:64], a_sb[:, gsl, 0:64], rzbb,
                        AL.mult)

                if stages == 5:
                    vdbg = ptl.tile([64, C], f32, tag="vdbg")
                    nc.vector.tensor_copy(vdbg[:], a_sb[0:64, 0:2, 0:64])
                    nc.sync.dma_start(out_d.ap()[n], vdbg[:])
                    continue

                # ---- VLAD matmul: [a | invmask]^T @ [xn | 1] ----
                vl = ps_vl.tile([65, 130], f32, tag="vl")
                for i in range(NT):
                    nc.tensor.matmul(
                        vl[:], a_sb[:, i, 0:65], xn[:, i, 0:130],
                        start=(i == 0), stop=(i == NT - 1))

                # ---- tail: uniform part, centroids, intra + global norm ----
                vrow = ptl.tile([1, 130], f32, tag="vrow")
                nc.scalar.activation(vrow[:], vl[64:65, :], AF.Copy)
                bps = ps_vl.tile([64, 130], f32, tag="vl")
                nc.tensor.matmul(bps[:], o1x64[:], vrow[:],
                                 start=True, stop=True)
                bsb = ptl.tile([64, 130], f32, tag="bsb")
                nc.scalar.activation(bsb[:], bps[:], AF.Copy)
                v1 = ptl.tile([64, 130], f32, tag="v1")
                nc.vector.scalar_tensor_tensor(
                    v1[:], bsb[:], 1.0 / 64.0, vl[0:64, :], AL.mult, AL.add)
                an = ptl.tile([64, 1], f32, tag="an")
                nc.vector.tensor_scalar(an[:], v1[:, 128:129], -1.0, None,
                                        AL.mult)
                v2 = ptl.tile([64, C], f32, tag="v2")
                nc.vector.scalar_tensor_tensor(
                    v2[:], cent[:], an[:, 0:1], v1[:, 0:128], AL.mult, AL.add)
                sqs = ptl.tile([64, C], f32, tag="sqs")
                nrm2 = ptl.tile([64, 1], f32, tag="nrm2")
                nc.vector.tensor_tensor(sqs[:], v2[:], v2[:], AL.mult)
                nc.vector.reduce_sum(nrm2[:], sqs[:],
                                     axis=mybir.AxisListType.X)
                # row unit-norm contribution: u = min(nrm2 * 1e24, 1)
                u_sb = ptl.tile([64, 1], f32, tag="u_sb")
                nc.vector.tensor_scalar(u_sb[:], nrm2[:], 1e24, 1.0,
                                        AL.mult, AL.min)
                rn = ptl.tile([64, 1], f32, tag="rn")
                nc.scalar.activation(rn[:], nrm2[:], AF.Sqrt,
                                     bias=eps24[0:64, 0:1])
                nc.vector.reciprocal(rn[:], rn[:])
                v3 = ptl.tile([64, C], f32, tag="v3")
                nc.vector.tensor_scalar(v3[:], v2[:], rn[:, 0:1], None,
                                        AL.mult)
                # global norm via PE: gn2 = u^T @ ones
                g2p = ps_vl.tile([1, 1], f32, tag="vl")
                nc.tensor.matmul(g2p[:], u_sb[:], o64[:],
                                 start=True, stop=True)
                gsq = ptl.tile([1, 1], f32, tag="gsq")
                nc.scalar.activation(gsq[:], g2p[:], AF.Sqrt,
                                     bias=eps24[0:1, 0:1])
                nc.vector.reciprocal(gsq[:], gsq[:])
                grb = ps_vl.tile([64, 1], f32, tag="vl")
                nc.tensor.matmul(grb[:], o1x64[:], gsq[:],
                                 start=True, stop=True)
                vout = ptl.tile([64, C], f32, tag="vout")
                nc.vector.tensor_scalar(vout[:], v3[:], grb[:, 0:1], None,
                                        AL.mult)
                nc.sync.dma_start(out_d.ap()[n], vout[:])

    nc.compile()
    return nc


def _host_prep(conv1_w, bn1_gamma, bn1_beta, bn1_mean, bn1_var,
               conv2_w, conv2_b, bn2_gamma, bn2_beta, bn2_mean, bn2_var,
               centroids):
    s1 = bn1_gamma / np.sqrt(bn1_var + EPS_BN)
    b1 = (bn1_beta - bn1_mean * s1).astype(np.float32).reshape(C, 1)
    w3 = conv1_w[:, 0, :, 1]              # [C, 3] taps
    d3 = np.zeros((3, 128, 128), np.float32)
    for d in range(3):
        np.fill_diagonal(d3[d], w3[:, d] * s1)
    s2 = bn2_gamma / np.sqrt(bn2_var + EPS_BN)
    w2 = conv2_w[:, :, 0, 0] * s2[:, None]        # [K, C]
    b2 = (s2 * conv2_b + bn2_beta - bn2_mean * s2).astype(np.float32)
    b2_hi = b2.astype(bfloat16)
    b2_lo = (b2 - b2_hi.astype(np.float32)).astype(bfloat16)
    rbm = np.zeros((10, 512), np.float32)
    rbm[0] = np.tile(b2_hi.astype(np.float32), 8)
    rbm[1] = np.tile(b2_lo.astype(np.float32), 8)
    for j in range(8):
        rbm[2 + j, 64 * j:64 * (j + 1)] = MASK_NEG
    i128 = np.eye(128, dtype=np.float32)
    return {
        "d3": d3.astype(bfloat16),
        "i128": i128.astype(bfloat16),
        "w2t": np.ascontiguousarray(w2.T).astype(bfloat16),
        "rb": rbm.astype(bfloat16),
        "b1": b1,
        "cent": np.ascontiguousarray(centroids.astype(np.float32)),
        "ones64": np.ones((64, 1), np.float32),
        "ones1x64": np.ones((1, 64), np.float32),
    }


def _mask_prep(length):
    t_idx = np.arange(T).reshape(NT, 128)          # [tile, p] -> t
    lbm = np.zeros((PB, NBK, 10, 128), np.float32)
    invm = np.zeros((PB, 128, NT), np.float32)
    for nn in range(PB):
        inv = (t_idx >= length[nn]).astype(np.float32)   # [NT, 128]
        invm[nn] = inv.T
        lbm[nn, :, 0, :] = 1.0
        lbm[nn, :, 1, :] = 1.0
        for g in range(NBK):
            for j in range(8):
                lbm[nn, g, 2 + j, :] = inv[8 * g + j]
    return lbm.astype(bfloat16), invm.astype(bfloat16)


def _get_nc():
    stages = int(os.environ.get("KSTAGES", "6"))
    if "nc" not in _CACHE:
        _CACHE["nc"] = _build_program(stages)
    return _CACHE["nc"]


def run(inputs, trace=False):
    nc = _get_nc()
    x_ = np.asarray(inputs["x_"], np.float32)
    length = np.asarray(inputs["length"])
    consts = _host_prep(
        np.asarray(inputs["conv1_w"], np.float32),
        np.asarray(inputs["bn1_gamma"], np.float32),
        np.asarray(inputs["bn1_beta"], np.float32),
        np.asarray(inputs["bn1_mean"], np.float32),
        np.asarray(inputs["bn1_var"], np.float32),
        np.asarray(inputs["conv2_w"], np.float32),
        np.asarray(inputs["conv2_b"], np.float32),
        np.asarray(inputs["bn2_gamma"], np.float32),
        np.asarray(inputs["bn2_beta"], np.float32),
        np.asarray(inputs["bn2_mean"], np.float32),
        np.asarray(inputs["bn2_var"], np.float32),
        np.asarray(inputs["centroids"], np.float32),
    )
    # [BS, T, C] -> per-core [PB, 128(p), NT(tile), 128(c)] contiguous
    xr = np.ascontiguousarray(
        x_.reshape(BS, NT, 128, C).transpose(0, 2, 1, 3))
    in_maps = []
    for ci in range(N_CORES):
        lbm, invm = _mask_prep(length[PB * ci:PB * (ci + 1)])
        m = dict(consts)
        m["x"] = np.ascontiguousarray(xr[PB * ci:PB * (ci + 1)])
        m["lb"] = lbm
        m["invm"] = invm
        in_maps.append(m)
    kw = {}
    if trace:
        kw = dict(trace=True)
    res = run_bass_kernel_spmd(nc, in_maps, core_ids=list(range(N_CORES)),
                               **kw)
    out = np.concatenate([res.results[ci]["out"].reshape(PB, K * C)
                          for ci in range(N_CORES)], axis=0)
    return out, res


def kernel(**inputs) -> np.ndarray:
    out, _ = run(inputs, trace=False)
    return out
